# revision 23
# baseline (speedup 1.0000x reference)
"""CrossAttention (single-head) Trainium2 kernel, 8-core data-parallel.

Full inputs in, full output out. Internally: batch 16 is sharded 2-per-core
across 8 NeuronCores; each core runs the whole attention layer for its two
batches in bf16 (f32 PSUM accumulation), with activations kept in transposed
[d, s] layout so every matmul contracts over the partition dim without any
on-chip transposes of large tensors.

v2 notes: PE streams at ~1.93-2.0 GHz sustained (P0 power state), back-to-back
with zero per-matmul overhead, so the only wins over the naive-dense schedule
are removing non-GEMM PE work: softmax column sums now ride a DVE f32
accumulation tree + one f32 ones-matmul per block (was 16 bf16 ones-matmuls),
and the 1/sums per-partition scatter uses a DRAM round-trip DMA instead of 4
PE transposes. PSUM pool deepened to 7 banks. Output stored bf16 (host
upcasts) to halve output DMA.
"""

import sys

sys.path.insert(0, "/opt/trn_rl_repo")

import numpy as np
import ml_dtypes

import concourse.bass as bass
import concourse.mybir as mybir
import concourse.tile as tile
from concourse.bass_utils import run_bass_kernel_spmd

BF16 = mybir.dt.bfloat16
F32 = mybir.dt.float32
AF = mybir.ActivationFunctionType

N_CORES = 8
B, S, D = 16, 2048, 1024
NB = B // N_CORES          # batches per core
KC = D // 128              # 8 chunks of 128 along d
ST = S // 128              # 16 tiles of 128 along s
NBLK = S // 512            # 4 blocks of 512 along s
SCALE = 1.0 / np.sqrt(np.float32(D))  # 1/32


def _split_waits(nc, limit=1):
    """Walrus in this container allows at most one sync wait per instruction:
    hoist excess waits onto NoOp carriers inserted just before."""
    n_new = 0
    for f in nc.m.functions:
        for bb in f.blocks:
            new_insts = []
            for inst in bb.instructions:
                si = inst.sync_info
                waits = list(si.on_wait) if si and si.on_wait else []
                if len(waits) > limit:
                    excess, keep = waits[:-limit], waits[-limit:]
                    for i in range(0, len(excess), limit):
                        chunk = excess[i:i + limit]
                        nop = mybir.InstNoOp(
                            name=f"{inst.name}-ws-{n_new}",
                            ins=[], outs=[],
                            sync_info=mybir.SyncInfo(on_wait=chunk, on_update=[]),
                        )
                        nop.engine = inst.engine
                        new_insts.append(nop)
                        n_new += 1
                    si.on_wait = keep
                new_insts.append(inst)
            bb.instructions[:] = new_insts
    return n_new



def _strip_dead_pe_updates(nc):
    """Drop PE sem increments nobody waits on (Tile emits one per matmul;
    only group-stop indices are ever waited). Renumber wait thresholds by
    rank among kept updates — release timing is identical, PE saves ~26ns
    per dropped serialized EVT_SEM write. Straight-line programs only."""
    pe = mybir.EngineType.PE
    insts = [i for f in nc.m.functions for bb in f.blocks for i in bb.instructions]
    upd_by_sem, wait_by_sem, bad = {}, {}, set()
    for inst in insts:
        si = inst.sync_info
        if not si:
            continue
        for u in (si.on_update or []):
            if u.sync_type != "semaphore":
                continue
            if inst.engine != pe or u.update_mode != "sem-inc" or u.update_value != 1:
                bad.add(u.id)
            upd_by_sem.setdefault(u.id, []).append((inst, u))
        for w in (si.on_wait or []):
            if w.sync_type != "semaphore":
                continue
            if w.wait_mode != "sem-ge-imm" or w.wait_reg is not None:
                bad.add(w.id)
            wait_by_sem.setdefault(w.id, []).append(w)
    n_drop = 0
    for sem_id, ups in upd_by_sem.items():
        if sem_id in bad or sem_id not in wait_by_sem or len(ups) < 16:
            continue
        waited = sorted({w.wait_value for w in wait_by_sem[sem_id]})
        if not waited or waited[-1] > len(ups) or waited[0] < 1:
            continue
        keep = set(waited)
        rank = {t: k + 1 for k, t in enumerate(waited)}
        for idx, (inst, u) in enumerate(ups, start=1):
            if idx not in keep:
                inst.sync_info.on_update = [
                    x for x in inst.sync_info.on_update if x is not u
                ]
                n_drop += 1
        for w in wait_by_sem[sem_id]:
            w.wait_value = rank[w.wait_value]
    return n_drop


def build_program(reps=1):
    """reps>1 wraps the whole computation in a hardware For_i loop — used
    only for timing (slope over reps isolates on-silicon exec time from
    per-call NEFF load overhead)."""
    nc = bass.Bass()

    qT_d = nc.declare_dram_parameter("qT", [NB, D, S], BF16, isOutput=False)
    kT_d = nc.declare_dram_parameter("kT", [NB, D, S], BF16, isOutput=False)
    vT_d = nc.declare_dram_parameter("vT", [NB, D, S], BF16, isOutput=False)
    Wq_d = nc.declare_dram_parameter("Wq", [D, D], BF16, isOutput=False)
    Wk_d = nc.declare_dram_parameter("Wk", [D, D], BF16, isOutput=False)
    Wv_d = nc.declare_dram_parameter("Wv", [D, D], BF16, isOutput=False)
    Wo_d = nc.declare_dram_parameter("Wo", [D, D], BF16, isOutput=False)
    # bq pre-scaled by 1/32 and reshaped [128, KC] host-side; bk likewise unscaled
    bq_d = nc.declare_dram_parameter("bq", [128, KC], F32, isOutput=False)
    bk_d = nc.declare_dram_parameter("bk", [128, KC], F32, isOutput=False)
    bv_d = nc.declare_dram_parameter("bv", [D], BF16, isOutput=False)
    bo_d = nc.declare_dram_parameter("bo", [D], BF16, isOutput=False)
    out_d = nc.declare_dram_parameter("out", [NB, S, D], BF16, isOutput=True)
    # per-(batch,blk) scratch for the [1,512] -> [128,4] r scatter
    rscr_d = nc.dram_tensor("rscr", [NB * NBLK, 512], F32)

    from contextlib import ExitStack
    with tile.TileContext(nc) as tc:
        with ExitStack() as _stk:
            _p = lambda **kw: _stk.enter_context(tc.tile_pool(**kw))
            wqopool = _p(name="wqo", bufs=8)
            wkvpool = _p(name="wkv", bufs=9)
            inpool = _p(name="inp", bufs=16)
            kpool = _p(name="keyT", bufs=8)
            vpool = _p(name="value", bufs=1)
            qpool = _p(name="queryT", bufs=12)
            epool = _p(name="expT", bufs=2)
            upool = _p(name="UT", bufs=2)
            opool = _p(name="outb", bufs=2)
            accpool = _p(name="acc", bufs=2)
            rpool = _p(name="rpool", bufs=1)
            cpool = _p(name="const", bufs=1)
            pspool = _p(name="ps", bufs=7, space="PSUM")
            ps1pool = _p(name="ps1", bufs=1, space="PSUM")
            # constants
            ones16 = cpool.tile([128, 1], mybir.dt.float16, tag="ones16")
            nc.vector.memset(ones16[:], 1.0)
            bq_sb = cpool.tile([128, KC], F32, tag="bq")
            nc.sync.dma_start(out=bq_sb[:], in_=bq_d[:])
            bk_sb = cpool.tile([128, KC], F32, tag="bk")
            nc.sync.dma_start(out=bk_sb[:], in_=bk_d[:])
            bv_sb = cpool.tile([128, D], BF16, tag="bv")
            ap = bv_d[:]
            nc.sync.dma_start(
                out=bv_sb[:],
                in_=bass.AP(tensor=ap.tensor, offset=ap.offset, ap=[[0, 128]] + ap.ap),
            )
            bo_sb = cpool.tile([128, D], BF16, tag="bo")
            ap = bo_d[:]
            nc.sync.dma_start(
                out=bo_sb[:],
                in_=bass.AP(tensor=ap.tensor, offset=ap.offset, ap=[[0, 128]] + ap.ap),
            )

            def load_w(w_d, pool, tag, slices=1):
                # slices>1 column-slices each tile's DMA so the first
                # consumer group (which only reads the first columns) isn't
                # gated on the whole 256KB transfer — used on the For_i
                # iteration-restart critical path only.
                cw = D // slices
                tiles = [
                    pool.tile([128, D], BF16, tag=tag, name=f"{tag}{i}")
                    for i in range(KC)
                ]
                # slice-major emission: the c=0 chunks of all tiles (what the
                # first matmul group reads) land on distinct DMA queues first
                for c in range(slices):
                    for i in range(KC):
                        nc.sync.dma_start(
                            out=tiles[i][:, c * cw:(c + 1) * cw],
                            in_=w_d[i * 128:(i + 1) * 128, c * cw:(c + 1) * cw],
                        )
                return tiles

            def load_in(src_d, b, i, s, slices=1):
                t = inpool.tile([128, 512], BF16, tag="inp", name=f"in{i}")
                cw = 512 // slices
                for c in range(slices):
                    nc.sync.dma_start(
                        out=t[:, c * cw:(c + 1) * cw],
                        in_=src_d[b, i * 128:(i + 1) * 128,
                                  s * 512 + c * cw:s * 512 + (c + 1) * cw],
                    )
                return t

            # Prologue prefetch (straight-line build only): the first keyT
            # group needs Wk + the first kin s-block, so enqueue those DMAs
            # ahead of the 4MB of Wq/Wo traffic. With a For_i timing loop the
            # wkv/inp ring slots must be (re)claimed inside the loop body, so
            # skip the hoist there — the slope metric amortizes the prologue.
            def load_restart(b):
                """keyT-phase loads for the start-of-body critical path (all
                DMA queues are empty there: kernel start, or just after the
                For_i reset barrier). Emission order puts the first matmul
                group's operands on distinct queues first: Wk column-0 slices,
                then the s=0 kin halves, then the remaining Wk columns."""
                wt = [
                    wkvpool.tile([128, D], BF16, tag="wkv", name=f"wkv{i}")
                    for i in range(KC)
                ]
                for i in range(KC):
                    nc.sync.dma_start(
                        out=wt[i][:, 0:256],
                        in_=Wk_d[i * 128:(i + 1) * 128, 0:256],
                    )
                kin = [
                    inpool.tile([128, 512], BF16, tag="inp", name=f"in{i}")
                    for i in range(KC)
                ]
                for i in range(KC):
                    for h in range(2):
                        nc.sync.dma_start(
                            out=kin[i][:, h * 256:(h + 1) * 256],
                            in_=kT_d[b, i * 128:(i + 1) * 128, h * 256:(h + 1) * 256],
                        )
                for c in range(1, 4):
                    for i in range(KC):
                        nc.sync.dma_start(
                            out=wt[i][:, c * 256:(c + 1) * 256],
                            in_=Wk_d[i * 128:(i + 1) * 128, c * 256:(c + 1) * 256],
                        )
                return wt, kin

            Wk_t0, kin00 = None, None
            if reps == 1:
                Wk_t0, kin00 = load_restart(0)

            # Wq and Wo stay resident for the whole kernel
            Wq_t = load_w(Wq_d, wqopool, "wq")
            Wo_t = load_w(Wo_d, wqopool, "wo")

            import contextlib
            loop_ctx = tc.For_i(0, reps, 1) if reps > 1 else contextlib.nullcontext()
            with loop_ctx:
              for b in range(NB):
                  # ---------------- keyT[d, s] = Wk.T @ kT (+bk) ----------------
                  kin0 = None
                  if b == 0:
                      if Wk_t0 is not None:
                          Wk_t, kin0 = Wk_t0, kin00
                      else:
                          Wk_t, kin0 = load_restart(0)
                  else:
                      Wk_t = load_w(Wk_d, wkvpool, "wkv")
                  keyT = [kpool.tile([128, S], BF16, tag="keyT", name=f"keyT{i}") for i in range(KC)]
                  for s in range(NBLK):
                      if s == 0 and kin0 is not None:
                          kin = kin0
                      else:
                          kin = [load_in(kT_d, b, i, s) for i in range(KC)]
                      for do in range(KC):
                          psum = pspool.tile([128, 512], F32, tag="ps")
                          for i in range(KC):
                              nc.tensor.matmul(
                                  psum[:], Wk_t[i][:, do * 128:(do + 1) * 128], kin[i][:],
                                  start=(i == 0), stop=(i == KC - 1),
                              )
                          nc.vector.tensor_scalar_add(
                              keyT[do][:, s * 512:(s + 1) * 512], psum[:],
                              bk_sb[:, do:do + 1],
                          )

                  # ---------------- value[s, d] = vT.T @ Wv (+bv) ----------------
                  Wv_t = load_w(Wv_d, wkvpool, "wkv")
                  val = vpool.tile([128, ST, D], BF16, tag="value")
                  for s in range(NBLK):
                      vin = [load_in(vT_d, b, i, s) for i in range(KC)]
                      for tt in range(4):
                          t16 = s * 4 + tt
                          for n in range(2):
                              psum = pspool.tile([128, 512], F32, tag="ps")
                              for i in range(KC):
                                  nc.tensor.matmul(
                                      psum[:],
                                      vin[i][:, tt * 128:(tt + 1) * 128],
                                      Wv_t[i][:, n * 512:(n + 1) * 512],
                                      start=(i == 0), stop=(i == KC - 1),
                                  )
                              nc.vector.tensor_add(
                                  val[:, t16, n * 512:(n + 1) * 512], psum[:],
                                  bv_sb[:, n * 512:(n + 1) * 512],
                              )

                  # ---------------- per 512-wide sq block ----------------
                  for blk in range(NBLK):
                      # queryT block [d, 512] = Wq.T @ qT_blk, scaled 1/32 (+bq/32)
                      qin = [load_in(qT_d, b, i, blk) for i in range(KC)]
                      qry = []
                      for do in range(KC):
                          psum = pspool.tile([128, 512], F32, tag="ps")
                          for i in range(KC):
                              nc.tensor.matmul(
                                  psum[:], Wq_t[i][:, do * 128:(do + 1) * 128], qin[i][:],
                                  start=(i == 0), stop=(i == KC - 1),
                              )
                          qt = qpool.tile([128, 512], BF16, tag="queryT", name=f"qry{do}")
                          nc.vector.tensor_scalar(
                              out=qt[:], in0=psum[:], scalar1=float(SCALE),
                              scalar2=bq_sb[:, do:do + 1],
                              op0=mybir.AluOpType.mult, op1=mybir.AluOpType.add,
                          )
                          qry.append(qt)

                      # scoresT -> expT; column sums accumulate on DVE in f32
                      exp_blk = epool.tile([128, ST, 512], BF16, tag="expT")
                      acc = accpool.tile([128, 512], F32, tag="acc")
                      for t16 in range(ST):
                          psum = pspool.tile([128, 512], F32, tag="ps")
                          for i in range(KC):
                              nc.tensor.matmul(
                                  psum[:],
                                  keyT[i][:, t16 * 128:(t16 + 1) * 128],
                                  qry[i][:],
                                  start=(i == 0), stop=(i == KC - 1),
                              )
                          nc.scalar.activation(exp_blk[:, t16, :], psum[:], AF.Exp)
                          if t16 == 1:
                              nc.vector.tensor_add(
                                  acc[:], exp_blk[:, 0, :], exp_blk[:, 1, :]
                              )
                          elif t16 > 1:
                              nc.vector.tensor_add(
                                  acc[:], acc[:], exp_blk[:, t16, :]
                              )

                      # UT block [d, 512] = value.T @ expT
                      ut = upool.tile([128, KC, 512], BF16, tag="UT")

                      def ut_group(j):
                          psum = pspool.tile([128, 512], F32, tag="ps")
                          for t16 in range(ST):
                              nc.tensor.matmul(
                                  psum[:],
                                  val[:, t16, j * 128:(j + 1) * 128],
                                  exp_blk[:, t16, :],
                                  start=(t16 == 0), stop=(t16 == ST - 1),
                              )
                          nc.vector.tensor_copy(ut[:, j, :], psum[:])

                      # j=0 first: its matmul stream hides the exp/acc tail
                      ut_group(0)

                      # column sums over all sk: one fp16 ones-matmul on the
                      # fp16-downcast acc (fp16 runs 1 cycle/row vs fp32's 4;
                      # sums ~3e3 with 2^-11 rel err, far inside budget)
                      acc16 = accpool.tile([128, 512], mybir.dt.float16, tag="acc16")
                      nc.vector.tensor_copy(acc16[:], acc[:])
                      sums_ps = ps1pool.tile([1, 512], F32, tag="ps1")
                      nc.tensor.matmul(
                          sums_ps[:], ones16[:], acc16[:], start=True, stop=True
                      )
                      r1 = rpool.tile([1, 512], F32, tag="r1")
                      nc.vector.reciprocal(r1[:], sums_ps[:])
                      # scatter r1[0, m*128+p] -> r_sb[p, m] via DRAM round-trip
                      sidx = b * NBLK + blk
                      nc.sync.dma_start(out=rscr_d[sidx, :], in_=r1[:])
                      r_sb = rpool.tile([128, 4], F32, tag="r")
                      sap = rscr_d[sidx, :]
                      nc.sync.dma_start(
                          out=r_sb[:],
                          in_=bass.AP(
                              tensor=sap.tensor, offset=sap.offset,
                              ap=[[1, 128], [128, 4]],
                          ),
                      )

                      for j in range(1, KC):
                          ut_group(j)

                      # final block: out[sq, d] = (UT.T @ Wo) * r + bo
                      for m in range(4):
                          ob = opool.tile([128, D], BF16, tag="outb")
                          for n in range(2):
                              psum = pspool.tile([128, 512], F32, tag="ps")
                              for j in range(KC):
                                  nc.tensor.matmul(
                                      psum[:],
                                      ut[:, j, m * 128:(m + 1) * 128],
                                      Wo_t[j][:, n * 512:(n + 1) * 512],
                                      start=(j == 0), stop=(j == KC - 1),
                                  )
                              nc.vector.scalar_tensor_tensor(
                                  out=ob[:, n * 512:(n + 1) * 512],
                                  in0=psum[:], scalar=r_sb[:, m:m + 1],
                                  in1=bo_sb[:, n * 512:(n + 1) * 512],
                                  op0=mybir.AluOpType.mult,
                                  op1=mybir.AluOpType.add,
                              )
                          # last store of the last block is the For_i
                          # iteration tail: slice it across queues so the
                          # end-of-iteration drain isn't gated on one ~11us
                          # 256KB transfer
                          sq = blk * 512 + m * 128
                          oslc = 1
                          if blk == NBLK - 1:
                              oslc = 4 if m == 3 else (2 if m == 2 else 1)
                          ocw = D // oslc
                          for c in range(oslc):
                              nc.sync.dma_start(
                                  out=out_d[b, sq:sq + 128, c * ocw:(c + 1) * ocw],
                                  in_=ob[:, c * ocw:(c + 1) * ocw],
                              )

    if reps == 1:
        _strip_dead_pe_updates(nc)
    _split_waits(nc)
    return nc


_PROGRAM = None


def _get_program():
    global _PROGRAM
    if _PROGRAM is None:
        _PROGRAM = build_program()
    return _PROGRAM


def prepare_in_maps(q, k, v, Wq, bq, Wk, bk, Wv, bv, Wo, bo):
    bf = ml_dtypes.bfloat16
    f32 = np.float32

    def t_bf16(x):  # [B,S,D] f32 -> [B,D,S] bf16 contiguous
        return np.ascontiguousarray(
            np.asarray(x, f32).astype(bf).transpose(0, 2, 1)
        )

    qT = t_bf16(q)
    kT = t_bf16(k)
    vT = t_bf16(v)
    Wq_b = np.asarray(Wq, f32).astype(bf)
    Wk_b = np.asarray(Wk, f32).astype(bf)
    Wv_b = np.asarray(Wv, f32).astype(bf)
    Wo_b = np.asarray(Wo, f32).astype(bf)
    bq2 = np.ascontiguousarray(
        (np.asarray(bq, f32) * np.float32(SCALE)).reshape(KC, 128).T
    )
    bk2 = np.ascontiguousarray(np.asarray(bk, f32).reshape(KC, 128).T)
    bv1 = np.ascontiguousarray(np.asarray(bv, f32)).astype(bf)
    bo1 = np.ascontiguousarray(np.asarray(bo, f32)).astype(bf)

    in_maps = []
    for c in range(N_CORES):
        sl = slice(c * NB, (c + 1) * NB)
        in_maps.append({
            "qT": qT[sl], "kT": kT[sl], "vT": vT[sl],
            "Wq": Wq_b, "Wk": Wk_b, "Wv": Wv_b, "Wo": Wo_b,
            "bq": bq2, "bk": bk2, "bv": bv1, "bo": bo1,
        })
    return in_maps


def kernel(q, k, v, Wq, bq, Wk, bk, Wv, bv, Wo, bo):
    nc = _get_program()
    in_maps = prepare_in_maps(q, k, v, Wq, bq, Wk, bk, Wv, bv, Wo, bo)
    res = run_bass_kernel_spmd(nc, in_maps, core_ids=list(range(N_CORES)))
    out = np.concatenate([res.results[c]["out"] for c in range(N_CORES)], axis=0)
    return out.astype(np.float32)


# revision 29
# speedup vs baseline: 1.0158x; 1.0158x over previous
"""CrossAttention (single-head) Trainium2 kernel, 8-core data-parallel.

Full inputs in, full output out. Internally: batch 16 is sharded 2-per-core
across 8 NeuronCores; each core runs the whole attention layer for its two
batches in bf16 (f32 PSUM accumulation), with activations kept in transposed
[d, s] layout so every matmul contracts over the partition dim without any
on-chip transposes of large tensors.

v2 notes: PE streams at ~1.93-2.0 GHz sustained (P0 power state), back-to-back
with zero per-matmul overhead, so the only wins over the naive-dense schedule
are removing non-GEMM PE work: softmax column sums now ride a DVE f32
accumulation tree + one f32 ones-matmul per block (was 16 bf16 ones-matmuls),
and the 1/sums per-partition scatter uses a DRAM round-trip DMA instead of 4
PE transposes. PSUM pool deepened to 7 banks. Output stored bf16 (host
upcasts) to halve output DMA.
"""

import sys

sys.path.insert(0, "/opt/trn_rl_repo")

import numpy as np
import ml_dtypes

import concourse.bass as bass
import concourse.mybir as mybir
import concourse.tile as tile
from concourse.bass_utils import run_bass_kernel_spmd

BF16 = mybir.dt.bfloat16
F32 = mybir.dt.float32
AF = mybir.ActivationFunctionType

N_CORES = 8
B, S, D = 16, 2048, 1024
NB = B // N_CORES          # batches per core
KC = D // 128              # 8 chunks of 128 along d
ST = S // 128              # 16 tiles of 128 along s
NBLK = S // 512            # 4 blocks of 512 along s
SCALE = 1.0 / np.sqrt(np.float32(D))  # 1/32


def _split_waits(nc, limit=1):
    """Walrus in this container allows at most one sync wait per instruction:
    hoist excess waits onto NoOp carriers inserted just before."""
    n_new = 0
    for f in nc.m.functions:
        for bb in f.blocks:
            new_insts = []
            for inst in bb.instructions:
                si = inst.sync_info
                waits = list(si.on_wait) if si and si.on_wait else []
                if len(waits) > limit:
                    excess, keep = waits[:-limit], waits[-limit:]
                    for i in range(0, len(excess), limit):
                        chunk = excess[i:i + limit]
                        nop = mybir.InstNoOp(
                            name=f"{inst.name}-ws-{n_new}",
                            ins=[], outs=[],
                            sync_info=mybir.SyncInfo(on_wait=chunk, on_update=[]),
                        )
                        nop.engine = inst.engine
                        new_insts.append(nop)
                        n_new += 1
                    si.on_wait = keep
                new_insts.append(inst)
            bb.instructions[:] = new_insts
    return n_new



def _strip_dead_pe_updates(nc):
    """Drop PE sem increments nobody waits on (Tile emits one per matmul;
    only group-stop indices are ever waited). Renumber wait thresholds by
    rank among kept updates — release timing is identical, PE saves ~26ns
    per dropped serialized EVT_SEM write. Straight-line programs only."""
    pe = mybir.EngineType.PE
    insts = [i for f in nc.m.functions for bb in f.blocks for i in bb.instructions]
    upd_by_sem, wait_by_sem, bad = {}, {}, set()
    for inst in insts:
        si = inst.sync_info
        if not si:
            continue
        for u in (si.on_update or []):
            if u.sync_type != "semaphore":
                continue
            if inst.engine != pe or u.update_mode != "sem-inc" or u.update_value != 1:
                bad.add(u.id)
            upd_by_sem.setdefault(u.id, []).append((inst, u))
        for w in (si.on_wait or []):
            if w.sync_type != "semaphore":
                continue
            if w.wait_mode != "sem-ge-imm" or w.wait_reg is not None:
                bad.add(w.id)
            wait_by_sem.setdefault(w.id, []).append(w)
    n_drop = 0
    for sem_id, ups in upd_by_sem.items():
        if sem_id in bad or sem_id not in wait_by_sem or len(ups) < 16:
            continue
        waited = sorted({w.wait_value for w in wait_by_sem[sem_id]})
        if not waited or waited[-1] > len(ups) or waited[0] < 1:
            continue
        keep = set(waited)
        rank = {t: k + 1 for k, t in enumerate(waited)}
        for idx, (inst, u) in enumerate(ups, start=1):
            if idx not in keep:
                inst.sync_info.on_update = [
                    x for x in inst.sync_info.on_update if x is not u
                ]
                n_drop += 1
        for w in wait_by_sem[sem_id]:
            w.wait_value = rank[w.wait_value]
    return n_drop


def build_program(reps=1, unroll=1):
    """reps>1 wraps the whole computation in a hardware For_i loop — used
    only for timing (slope over reps isolates on-silicon exec time from
    per-call NEFF load overhead). unroll>1 puts that many kernel copies in
    the loop body (diagnostic: separates the per-iteration reset-barrier
    bubble from the true pipelined kernel cost)."""
    nc = bass.Bass()

    qT_d = nc.declare_dram_parameter("qT", [NB, D, S], BF16, isOutput=False)
    kT_d = nc.declare_dram_parameter("kT", [NB, D, S], BF16, isOutput=False)
    vT_d = nc.declare_dram_parameter("vT", [NB, D, S], BF16, isOutput=False)
    Wq_d = nc.declare_dram_parameter("Wq", [D, D], BF16, isOutput=False)
    Wk_d = nc.declare_dram_parameter("Wk", [D, D], BF16, isOutput=False)
    Wv_d = nc.declare_dram_parameter("Wv", [D, D], BF16, isOutput=False)
    Wo_d = nc.declare_dram_parameter("Wo", [D, D], BF16, isOutput=False)
    # bq pre-scaled by 1/32 and reshaped [128, KC] host-side; bk likewise unscaled
    bq_d = nc.declare_dram_parameter("bq", [128, KC], F32, isOutput=False)
    bk_d = nc.declare_dram_parameter("bk", [128, KC], F32, isOutput=False)
    bv_d = nc.declare_dram_parameter("bv", [D], BF16, isOutput=False)
    bo_d = nc.declare_dram_parameter("bo", [D], BF16, isOutput=False)
    out_d = nc.declare_dram_parameter("out", [NB, S, D], BF16, isOutput=True)
    # per-(batch,blk) scratch for the [1,512] -> [128,4] r scatter
    rscr_d = nc.dram_tensor("rscr", [NB * NBLK, 512], F32)

    from contextlib import ExitStack
    with tile.TileContext(nc) as tc:
        with ExitStack() as _stk:
            _p = lambda **kw: _stk.enter_context(tc.tile_pool(**kw))
            wqopool = _p(name="wqo", bufs=8)
            wkvpool = _p(name="wkv", bufs=16)
            inpool = _p(name="inp", bufs=14)
            kpool = _p(name="keyT", bufs=8)
            vpool = _p(name="value", bufs=1)
            qpool = _p(name="queryT", bufs=10)
            epool = _p(name="expT", bufs=2)
            upool = _p(name="UT", bufs=1)
            opool = _p(name="outb", bufs=2)
            accpool = _p(name="acc", bufs=1)
            rpool = _p(name="rpool", bufs=1)
            cpool = _p(name="const", bufs=1)
            pspool = _p(name="ps", bufs=7, space="PSUM")
            ps1pool = _p(name="ps1", bufs=1, space="PSUM")
            # constants
            ones16 = cpool.tile([128, 1], mybir.dt.float16, tag="ones16")
            nc.vector.memset(ones16[:], 1.0)
            bq_sb = cpool.tile([128, KC], F32, tag="bq")
            nc.sync.dma_start(out=bq_sb[:], in_=bq_d[:])
            bk_sb = cpool.tile([128, KC], F32, tag="bk")
            nc.sync.dma_start(out=bk_sb[:], in_=bk_d[:])
            bv_sb = cpool.tile([128, D], BF16, tag="bv")
            ap = bv_d[:]
            nc.sync.dma_start(
                out=bv_sb[:],
                in_=bass.AP(tensor=ap.tensor, offset=ap.offset, ap=[[0, 128]] + ap.ap),
            )
            bo_sb = cpool.tile([128, D], BF16, tag="bo")
            ap = bo_d[:]
            nc.sync.dma_start(
                out=bo_sb[:],
                in_=bass.AP(tensor=ap.tensor, offset=ap.offset, ap=[[0, 128]] + ap.ap),
            )

            def load_w(w_d, pool, tag, slices=1):
                # slices>1 column-slices each tile's DMA so the first
                # consumer group (which only reads the first columns) isn't
                # gated on the whole 256KB transfer — used on the For_i
                # iteration-restart critical path only.
                cw = D // slices
                tiles = [
                    pool.tile([128, D], BF16, tag=tag, name=f"{tag}{i}")
                    for i in range(KC)
                ]
                # slice-major emission: the c=0 chunks of all tiles (what the
                # first matmul group reads) land on distinct DMA queues first
                for c in range(slices):
                    for i in range(KC):
                        nc.sync.dma_start(
                            out=tiles[i][:, c * cw:(c + 1) * cw],
                            in_=w_d[i * 128:(i + 1) * 128, c * cw:(c + 1) * cw],
                        )
                return tiles

            def load_in(src_d, b, i, s, slices=1):
                t = inpool.tile([128, 512], BF16, tag="inp", name=f"in{i}")
                cw = 512 // slices
                for c in range(slices):
                    nc.sync.dma_start(
                        out=t[:, c * cw:(c + 1) * cw],
                        in_=src_d[b, i * 128:(i + 1) * 128,
                                  s * 512 + c * cw:s * 512 + (c + 1) * cw],
                    )
                return t

            # Prologue prefetch (straight-line build only): the first keyT
            # group needs Wk + the first kin s-block, so enqueue those DMAs
            # ahead of the 4MB of Wq/Wo traffic. With a For_i timing loop the
            # wkv/inp ring slots must be (re)claimed inside the loop body, so
            # skip the hoist there — the slope metric amortizes the prologue.
            def load_restart(b):
                """keyT-phase loads for the start-of-body critical path (all
                DMA queues are empty there: kernel start, or just after the
                For_i reset barrier). Emission order puts the first matmul
                group's operands on distinct queues first: Wk column-0 slices,
                then the s=0 kin halves, then the remaining Wk columns."""
                wt = [
                    wkvpool.tile([128, D], BF16, tag="wkv", name=f"wkv{i}")
                    for i in range(KC)
                ]
                for i in range(KC):
                    nc.sync.dma_start(
                        out=wt[i][:, 0:256],
                        in_=Wk_d[i * 128:(i + 1) * 128, 0:256],
                    )
                kin = [
                    inpool.tile([128, 512], BF16, tag="inp", name=f"in{i}")
                    for i in range(KC)
                ]
                for i in range(KC):
                    for h in range(2):
                        nc.sync.dma_start(
                            out=kin[i][:, h * 256:(h + 1) * 256],
                            in_=kT_d[b, i * 128:(i + 1) * 128, h * 256:(h + 1) * 256],
                        )
                for c in range(1, 4):
                    for i in range(KC):
                        nc.sync.dma_start(
                            out=wt[i][:, c * 256:(c + 1) * 256],
                            in_=Wk_d[i * 128:(i + 1) * 128, c * 256:(c + 1) * 256],
                        )
                return wt, kin

            # ALL weights stay resident for the whole kernel (loaded once in
            # the prologue): weights are invocation-invariant, and dropping
            # the per-batch Wk/Wv reloads removes 8MB/rep of DMA traffic —
            # the kernel's sustained rate runs measurably behind an
            # equivalent DMA-free matmul stream (power-state coupling), and
            # the iteration restart no longer waits on any weight DMA.
            kin00 = None
            if reps == 1:
                Wk_t, kin00 = load_restart(0)
            else:
                Wk_t = load_w(Wk_d, wkvpool, "wkv", slices=4)
            Wv_t = load_w(Wv_d, wkvpool, "wkv")
            Wq_t = load_w(Wq_d, wqopool, "wq")
            Wo_t = load_w(Wo_d, wqopool, "wo")

            import contextlib
            loop_ctx = tc.For_i(0, reps, 1) if reps > 1 else contextlib.nullcontext()
            with loop_ctx:
             for u in range(unroll):
              for b in range(NB):
                  # ---------------- keyT[d, s] = Wk.T @ kT (+bk) ----------------
                  keyT = [kpool.tile([128, S], BF16, tag="keyT", name=f"keyT{i}") for i in range(KC)]
                  for s in range(NBLK):
                      if u == 0 and b == 0 and s == 0:
                          # iteration-restart critical path (queues empty
                          # after the For_i reset): halved kin transfers
                          # spread the first group's wait across queues
                          kin = kin00 if kin00 is not None else [
                              load_in(kT_d, 0, i, 0, slices=2) for i in range(KC)
                          ]
                      else:
                          kin = [load_in(kT_d, b, i, s) for i in range(KC)]
                      for do in range(KC):
                          psum = pspool.tile([128, 512], F32, tag="ps")
                          for i in range(KC):
                              nc.tensor.matmul(
                                  psum[:], Wk_t[i][:, do * 128:(do + 1) * 128], kin[i][:],
                                  start=(i == 0), stop=(i == KC - 1),
                              )
                          nc.vector.tensor_scalar_add(
                              keyT[do][:, s * 512:(s + 1) * 512], psum[:],
                              bk_sb[:, do:do + 1],
                          )

                  # ---------------- value[s, d] = vT.T @ Wv (+bv) ----------------
                  val = vpool.tile([128, ST, D], BF16, tag="value")
                  for s in range(NBLK):
                      vin = [load_in(vT_d, b, i, s) for i in range(KC)]
                      for tt in range(4):
                          t16 = s * 4 + tt
                          for n in range(2):
                              psum = pspool.tile([128, 512], F32, tag="ps")
                              for i in range(KC):
                                  nc.tensor.matmul(
                                      psum[:],
                                      vin[i][:, tt * 128:(tt + 1) * 128],
                                      Wv_t[i][:, n * 512:(n + 1) * 512],
                                      start=(i == 0), stop=(i == KC - 1),
                                  )
                              nc.vector.tensor_add(
                                  val[:, t16, n * 512:(n + 1) * 512], psum[:],
                                  bv_sb[:, n * 512:(n + 1) * 512],
                              )

                  # ---------------- per 512-wide sq block ----------------
                  for blk in range(NBLK):
                      # queryT block [d, 512] = Wq.T @ qT_blk, scaled 1/32 (+bq/32)
                      qin = [load_in(qT_d, b, i, blk) for i in range(KC)]
                      qry = []
                      for do in range(KC):
                          psum = pspool.tile([128, 512], F32, tag="ps")
                          for i in range(KC):
                              nc.tensor.matmul(
                                  psum[:], Wq_t[i][:, do * 128:(do + 1) * 128], qin[i][:],
                                  start=(i == 0), stop=(i == KC - 1),
                              )
                          qt = qpool.tile([128, 512], BF16, tag="queryT", name=f"qry{do}")
                          nc.vector.tensor_scalar(
                              out=qt[:], in0=psum[:], scalar1=float(SCALE),
                              scalar2=bq_sb[:, do:do + 1],
                              op0=mybir.AluOpType.mult, op1=mybir.AluOpType.add,
                          )
                          qry.append(qt)

                      # scoresT -> expT; column sums accumulate on DVE in f32
                      exp_blk = epool.tile([128, ST, 512], BF16, tag="expT")
                      acc = accpool.tile([128, 512], F32, tag="acc")
                      for t16 in range(ST):
                          psum = pspool.tile([128, 512], F32, tag="ps")
                          for i in range(KC):
                              nc.tensor.matmul(
                                  psum[:],
                                  keyT[i][:, t16 * 128:(t16 + 1) * 128],
                                  qry[i][:],
                                  start=(i == 0), stop=(i == KC - 1),
                              )
                          nc.scalar.activation(exp_blk[:, t16, :], psum[:], AF.Exp)
                          if t16 == 1:
                              nc.vector.tensor_add(
                                  acc[:], exp_blk[:, 0, :], exp_blk[:, 1, :]
                              )
                          elif t16 > 1:
                              nc.vector.tensor_add(
                                  acc[:], acc[:], exp_blk[:, t16, :]
                              )

                      # UT block [d, 512] = value.T @ expT
                      ut = upool.tile([128, KC, 512], BF16, tag="UT")

                      def ut_group(j):
                          psum = pspool.tile([128, 512], F32, tag="ps")
                          for t16 in range(ST):
                              nc.tensor.matmul(
                                  psum[:],
                                  val[:, t16, j * 128:(j + 1) * 128],
                                  exp_blk[:, t16, :],
                                  start=(t16 == 0), stop=(t16 == ST - 1),
                              )
                          nc.vector.tensor_copy(ut[:, j, :], psum[:])

                      # j=0 first: its matmul stream hides the exp/acc tail
                      ut_group(0)

                      # column sums over all sk: one fp16 ones-matmul on the
                      # fp16-downcast acc (fp16 runs 1 cycle/row vs fp32's 4;
                      # sums ~3e3 with 2^-11 rel err, far inside budget)
                      acc16 = accpool.tile([128, 512], mybir.dt.float16, tag="acc16")
                      nc.vector.tensor_copy(acc16[:], acc[:])
                      sums_ps = ps1pool.tile([1, 512], F32, tag="ps1")
                      nc.tensor.matmul(
                          sums_ps[:], ones16[:], acc16[:], start=True, stop=True
                      )
                      r1 = rpool.tile([1, 512], F32, tag="r1")
                      nc.vector.reciprocal(r1[:], sums_ps[:])
                      # scatter r1[0, m*128+p] -> r_sb[p, m] via DRAM round-trip
                      sidx = b * NBLK + blk
                      nc.sync.dma_start(out=rscr_d[sidx, :], in_=r1[:])
                      r_sb = rpool.tile([128, 4], F32, tag="r")
                      sap = rscr_d[sidx, :]
                      nc.sync.dma_start(
                          out=r_sb[:],
                          in_=bass.AP(
                              tensor=sap.tensor, offset=sap.offset,
                              ap=[[1, 128], [128, 4]],
                          ),
                      )

                      for j in range(1, KC):
                          ut_group(j)

                      # final block: out[sq, d] = (UT.T @ Wo) * r + bo
                      for m in range(4):
                          ob = opool.tile([128, D], BF16, tag="outb")
                          for n in range(2):
                              psum = pspool.tile([128, 512], F32, tag="ps")
                              for j in range(KC):
                                  nc.tensor.matmul(
                                      psum[:],
                                      ut[:, j, m * 128:(m + 1) * 128],
                                      Wo_t[j][:, n * 512:(n + 1) * 512],
                                      start=(j == 0), stop=(j == KC - 1),
                                  )
                              nc.vector.scalar_tensor_tensor(
                                  out=ob[:, n * 512:(n + 1) * 512],
                                  in0=psum[:], scalar=r_sb[:, m:m + 1],
                                  in1=bo_sb[:, n * 512:(n + 1) * 512],
                                  op0=mybir.AluOpType.mult,
                                  op1=mybir.AluOpType.add,
                              )
                          # last store of the last block is the For_i
                          # iteration tail: slice it across queues so the
                          # end-of-iteration drain isn't gated on one ~11us
                          # 256KB transfer
                          sq = blk * 512 + m * 128
                          oslc = 1
                          if blk == NBLK - 1:
                              oslc = 4 if m == 3 else (2 if m == 2 else 1)
                          ocw = D // oslc
                          for c in range(oslc):
                              nc.sync.dma_start(
                                  out=out_d[b, sq:sq + 128, c * ocw:(c + 1) * ocw],
                                  in_=ob[:, c * ocw:(c + 1) * ocw],
                              )

    if reps == 1:
        _strip_dead_pe_updates(nc)
    _split_waits(nc)
    return nc


_PROGRAM = None


def _get_program():
    global _PROGRAM
    if _PROGRAM is None:
        _PROGRAM = build_program()
    return _PROGRAM


def prepare_in_maps(q, k, v, Wq, bq, Wk, bk, Wv, bv, Wo, bo):
    bf = ml_dtypes.bfloat16
    f32 = np.float32

    def t_bf16(x):  # [B,S,D] f32 -> [B,D,S] bf16 contiguous
        return np.ascontiguousarray(
            np.asarray(x, f32).astype(bf).transpose(0, 2, 1)
        )

    qT = t_bf16(q)
    kT = t_bf16(k)
    vT = t_bf16(v)
    Wq_b = np.asarray(Wq, f32).astype(bf)
    Wk_b = np.asarray(Wk, f32).astype(bf)
    Wv_b = np.asarray(Wv, f32).astype(bf)
    Wo_b = np.asarray(Wo, f32).astype(bf)
    bq2 = np.ascontiguousarray(
        (np.asarray(bq, f32) * np.float32(SCALE)).reshape(KC, 128).T
    )
    bk2 = np.ascontiguousarray(np.asarray(bk, f32).reshape(KC, 128).T)
    bv1 = np.ascontiguousarray(np.asarray(bv, f32)).astype(bf)
    bo1 = np.ascontiguousarray(np.asarray(bo, f32)).astype(bf)

    in_maps = []
    for c in range(N_CORES):
        sl = slice(c * NB, (c + 1) * NB)
        in_maps.append({
            "qT": qT[sl], "kT": kT[sl], "vT": vT[sl],
            "Wq": Wq_b, "Wk": Wk_b, "Wv": Wv_b, "Wo": Wo_b,
            "bq": bq2, "bk": bk2, "bv": bv1, "bo": bo1,
        })
    return in_maps


def kernel(q, k, v, Wq, bq, Wk, bk, Wv, bv, Wo, bo):
    nc = _get_program()
    in_maps = prepare_in_maps(q, k, v, Wq, bq, Wk, bk, Wv, bv, Wo, bo)
    res = run_bass_kernel_spmd(nc, in_maps, core_ids=list(range(N_CORES)))
    out = np.concatenate([res.results[c]["out"] for c in range(N_CORES)], axis=0)
    return out.astype(np.float32)
